# revision 23
# baseline (speedup 1.0000x reference)
"""Trainium2 Bass kernel for int8-dequant Linear: out = x @ (W_q * scaler)^T.

Full shapes: x [4, 2048, 4096] f32, weight_q [4096, 4096] int8,
weight_scaler [4096] f32 -> out [4, 2048, 4096] f32.

Sharding: data-parallel over tokens (8192 tokens -> 1024 per core);
weight_q/scaler replicated. Each core computes out.T for its token
shard with out-channels on PSUM partitions; the per-channel scaler is
applied as a per-partition scalar multiply on PSUM eviction (output
rounds to fp16: adds <6e-3 rel err, halves the out DMA and tail).

Matmul dtype (MODE):
  "fp16" (default): x rounded to fp16 on host (rel err ~2^-12); W_q is
    exactly representable in fp16 (int8 values). fp16 weights enable
    FWL (fast weight load, ~75ns) so the per-matmul LDWEIGHTS fully
    hides under the 512-cycle moving stream; measured MM issue gap is
    ~216.6ns vs the 213.3ns streaming floor. W ships as int8 (head
    DMA is bandwidth-bound; int8 halves critical-path bytes) and is
    upconverted by the otherwise-idle DVE.
  "f32r": x f32 truncated to ~fp22 in the PE. ~20us slower: f32r
    forbids standalone LDWEIGHTS; its self-load costs ~14ns per MM.

Head: DMA issue slices cost ~0.6us each on their issuing engine queue
and early transfers share ~250GB/s of DMA bandwidth, so the first
tiles are small and spread across queues: a 2-k-slice weight sliver
(32KB) plus x halves so the first matmul gates on ~160KB, with issues
split across the Sync/Scalar/GpSimd queues. A short burst of tiny
warm-up matmuls precedes the first real one (best-measured variant;
HAM-throttle behavior makes bigger warm-ups counterproductive).
Tail: the last mo-tile runs n-outer so only one PSUM bank drains
after the final matmul; its eviction is split into two chunks whose
output DMAs issue from different queues.
"""
import sys

sys.path.insert(0, "/opt/trn_rl_repo")

import numpy as np

import concourse.bacc as bacc
import concourse.mybir as mybir
import concourse.tile as tile
from concourse.bass_utils import run_bass_kernel_spmd

N_CORES = 8
P = 128
IN_F = 4096
OUT_F = 4096
TOKENS = 4 * 2048
T_SHARD = TOKENS // N_CORES          # 1024 tokens per core
KT = IN_F // P                       # 32 k-tiles
MT = OUT_F // P                      # 32 m-tiles (out-channel tiles)
N_FREE = 512                         # moving free dim per matmul (1 PSUM bank)
NT = T_SHARD // N_FREE               # 2 n-tiles

MODE = "fp16"                        # "fp16" | "f32r"

_cache = {}


def _build(mode):
    f32 = mybir.dt.float32
    mm_dt = {
        "fp16": mybir.dt.float16,
        "f32r": mybir.dt.float32r,
    }[mode]
    out_dt = mybir.dt.float16 if mode == "fp16" else f32

    nc = bacc.Bacc(None, target_bir_lowering=False, debug=False)

    d_x = nc.declare_dram_parameter("xq0", [IN_F, T_SHARD], mm_dt, isOutput=False)
    d_w = nc.declare_dram_parameter("wq4", [MT, P, KT, P], mybir.dt.int8, isOutput=False)
    d_s = nc.declare_dram_parameter("scal", [P, MT], f32, isOutput=False)
    d_o = nc.declare_dram_parameter("outT", [MT, P, T_SHARD], out_dt, isOutput=True)

    WH = 8                       # k-tiles per weight quarter-tile
    NH = KT // WH                # weight sub-tiles per mo
    PRO = 4                      # mo-tiles interleaved during the x load
    KA = 2                       # k-slices in the fast first weight sliver

    with tile.TileContext(nc) as tc:
        with (
            tc.tile_pool(name="xp", bufs=KT) as xp,
            tc.tile_pool(name="wp", bufs=12) as wp,
            tc.tile_pool(name="ws", bufs=6) as ws,
            tc.tile_pool(name="op", bufs=4) as op,
            tc.tile_pool(name="cp", bufs=1) as cp,
            tc.tile_pool(name="ps", bufs=8, space="PSUM") as ps,
        ):
            # PE warm-up bridge: 10 dummy matmuls, structurally identical
            # to the real ones ([P,128] stationary, [P,512] moving,
            # [128,512] PSUM out), keep the PE at 100% duty from ~7.8us
            # until the real stream's data is staged (~12.2us). The HAM
            # clock-gate un-throttles DURING the bridge, so the reals run
            # warm and stall-free every time instead of gambling on DMA
            # arrival order. (Tiny N=16 dummies don't register as
            # HAM-busy; a partial-partition [16,512] PSUM out here
            # intermittently crashed the exec unit - full 128-partition
            # out matches the shape the kernel runs 2048 times anyway.)
            dmw = cp.tile([P, P], mm_dt, tag="dmw", name="dmw")
            dmy = cp.tile([P, N_FREE], mm_dt, tag="dmy", name="dmy")
            nc.gpsimd.memset(dmw[:], 0)
            nc.gpsimd.memset(dmy[:], 0)
            dps = ps.tile([P, N_FREE], f32, tag="psum", name="dummy_ps")
            for _ in range(10):
                nc.tensor.matmul(dps[:], dmw[:], dmy[:], start=True, stop=True)

            scal = cp.tile([P, MT], f32, tag="scal", name="scal")

            def w_part(mo, k0, nk, eng, tagsfx):
                s = ws.tile(
                    [P, nk, P], mybir.dt.int8,
                    tag=f"w8{tagsfx}", bufs=(6 if not tagsfx else 4),
                    name=f"w8{tagsfx}_{mo}_{k0}",
                )
                eng.dma_start(s[:], d_w.ap()[mo, :, k0:k0 + nk, :])
                t = wp.tile(
                    [P, nk, P], mm_dt,
                    tag=f"wh{tagsfx}", bufs=(12 if not tagsfx else 4),
                    name=f"wh{tagsfx}_{mo}_{k0}",
                )
                nc.vector.tensor_copy(t[:], s[:])
                return t

            def x_tile(k):
                t = xp.tile([P, T_SHARD], mm_dt, tag="xt", name=f"xt_{k}")
                nc.sync.dma_start(t[:], d_x.ap()[k * P:(k + 1) * P, :])
                return t

            def evict(mo, n, psum, split=False):
                # split=True pipelines DVE-evict with DMA issue on two
                # queues for the final tile (shortens the tail).
                n_chunk = 2 if split else 1
                c = N_FREE // n_chunk
                engs = [nc.sync, nc.scalar]
                for j in range(n_chunk):
                    osb = op.tile([P, c], out_dt, tag="osb", name=f"osb_{mo}_{n}_{j}")
                    nc.vector.tensor_scalar_mul(
                        osb[:], psum[:, j * c:(j + 1) * c], scal[:, mo:mo + 1]
                    )
                    engs[j % 2].dma_start(
                        d_o.ap()[mo, :, n * N_FREE + j * c:n * N_FREE + (j + 1) * c],
                        osb[:],
                    )

            xt = [None] * KT
            x0h = [None] * NT
            wh_a = {}                # (mo) -> [P, KA, P] sliver, k=0..KA-1
            wh_b = {}                # (mo) -> [P, WH-KA, P], k=KA..WH-1
            wh_pro = {}              # (mo, h>=1) -> [P, WH, P]
            wh_next = {}

            def w_slice(mo, k):
                if mo < PRO and k < WH:
                    if k < KA:
                        return wh_a[mo][:, k, :]
                    return wh_b[mo][:, k - KA, :]
                h, kh = divmod(k, WH)
                if mo < PRO:
                    return wh_pro[(mo, h)][:, kh, :]
                raise KeyError

            def x_slice(k, n):
                if k == 0:
                    return x0h[n][:]
                return xt[k][:, n * N_FREE:(n + 1) * N_FREE]

            # --- phase 1 head: smallest-first, spread across queues.
            wh_a[0] = w_part(0, 0, KA, nc.scalar, "a")
            for n in range(NT):
                t = xp.tile([P, N_FREE], mm_dt, tag="x0h", bufs=NT, name=f"x0h_{n}")
                nc.sync.dma_start(t[:], d_x.ap()[0:P, n * N_FREE:(n + 1) * N_FREE])
                x0h[n] = t
            for mo in range(1, PRO):
                wh_a[mo] = w_part(mo, 0, KA, nc.gpsimd, "a")
            for mo in range(PRO):
                wh_b[mo] = w_part(mo, KA, WH - KA, nc.scalar, "b")
            for i in range(1, WH):
                xt[i] = x_tile(i)
            nc.scalar.dma_start(scal[:], d_s.ap())
            for h in range(1, NH):
                for mo in range(PRO):
                    wh_pro[(mo, h)] = w_part(mo, h * WH, WH, nc.scalar, "")
                if h == NH - 1:
                    for j in range(NH):
                        wh_next[j] = w_part(PRO, j * WH, WH, nc.scalar, "")
                for i in range(h * WH, (h + 1) * WH):
                    xt[i] = x_tile(i)

            pro_ps = {
                (mo, n): ps.tile(
                    [P, N_FREE], f32, tag="psum", name=f"psum_{mo}_{n}"
                )
                for mo in range(PRO)
                for n in range(NT)
            }
            # Request mo=PRO's banks now so the allocator binds them to
            # the earliest-released phase-1 banks.
            early_ps = [
                ps.tile([P, N_FREE], f32, tag="psum", name=f"psum_{PRO}_{n}")
                for n in range(NT)
            ]
            # k-major across mo so the PE has work for every x k-tile as
            # it lands.
            for k in range(KT):
                for mo in range(PRO):
                    for n in range(NT):
                        nc.tensor.matmul(
                            pro_ps[(mo, n)][:],
                            w_slice(mo, k),
                            x_slice(k, n),
                            start=(k == 0),
                            stop=(k == KT - 1),
                        )
            for mo in range(PRO):
                for n in range(NT):
                    evict(mo, n, pro_ps[(mo, n)])

            # --- phase 2: remaining mo-tiles, weight-reuse-friendly order
            # (k middle, n inner). The last mo runs n-outer/k-inner so its
            # n=0 bank evicts ~7us before the final matmul, leaving only
            # the n=1 evict+DMA on the tail.
            for mo in range(PRO, MT):
                if mo == PRO and wh_next:
                    whs = [wh_next[h] for h in range(NH)]
                else:
                    whs = [w_part(mo, h * WH, WH, nc.scalar, "") for h in range(NH)]
                if mo == PRO:
                    psums = early_ps
                else:
                    psums = [
                        ps.tile([P, N_FREE], f32, tag="psum", name=f"psum_{mo}_{n}")
                        for n in range(NT)
                    ]
                if mo < MT - 1:
                    for k in range(KT):
                        h, kh = divmod(k, WH)
                        for n in range(NT):
                            nc.tensor.matmul(
                                psums[n][:],
                                whs[h][:, kh, :],
                                x_slice(k, n),
                                start=(k == 0),
                                stop=(k == KT - 1),
                            )
                    for n in range(NT):
                        evict(mo, n, psums[n])
                else:
                    for n in range(NT):
                        for k in range(KT):
                            h, kh = divmod(k, WH)
                            nc.tensor.matmul(
                                psums[n][:],
                                whs[h][:, kh, :],
                                x_slice(k, n),
                                start=(k == 0),
                                stop=(k == KT - 1),
                            )
                        evict(mo, n, psums[n], split=(n == NT - 1))

    nc.compile()
    return nc


def _prep_inputs(x, weight_q, weight_scaler, mode):
    """Host-side shard + layout. Returns in_maps (list of dicts, one per core)."""
    xf = np.asarray(x, dtype=np.float32).reshape(TOKENS, IN_F)
    wq = np.asarray(weight_q)
    sc = np.asarray(weight_scaler, dtype=np.float32)

    # W tiles: w4[mo, p_in, ko, oc] = W[mo*128+oc, ko*128+p_in]
    # (matches the SBUF lhsT tile AP [P, KT, P] exactly), shipped as int8
    # and upconverted on-chip.
    w4 = np.ascontiguousarray(
        wq.reshape(MT, P, KT, P).transpose(0, 3, 2, 1)
    ).astype(np.int8)

    scal = np.ascontiguousarray(sc.reshape(MT, P).T)  # [P, MT]

    in_maps = []
    for c in range(N_CORES):
        xs = xf[c * T_SHARD:(c + 1) * T_SHARD, :]      # [T_SHARD, IN_F]
        xsT = np.ascontiguousarray(xs.T)                # [IN_F, T_SHARD] f32
        if mode == "fp16":
            xsT = xsT.astype(np.float16)
        in_maps.append({"wq4": w4, "scal": scal, "xq0": xsT})
    return in_maps


def _gather(results):
    """Per-core outT [MT, P, T_SHARD] -> full out [4, 2048, OUT_F] f32."""
    parts = []
    for c in range(N_CORES):
        ot = results[c]["outT"]                   # [MT, P, T_SHARD]
        parts.append(ot.reshape(OUT_F, T_SHARD).T)  # [T_SHARD, OUT_F]
    out = np.concatenate(parts, axis=0)           # [TOKENS, OUT_F]
    return np.ascontiguousarray(
        out.reshape(4, 2048, OUT_F).astype(np.float32)
    )


def _run(inputs, trace=False, mode=None):
    mode = mode or MODE
    if mode not in _cache:
        _cache[mode] = _build(mode)
    nc = _cache[mode]
    in_maps = _prep_inputs(inputs["x"], inputs["weight_q"], inputs["weight_scaler"], mode)
    res = run_bass_kernel_spmd(nc, in_maps, list(range(N_CORES)), trace=trace)
    return _gather(res.results), res


def kernel(**inputs):
    out, _ = _run(inputs, trace=False)
    return out
